# revision 15
# baseline (speedup 1.0000x reference)
"""Trainium2 Bass kernel for nn_BaseGNNModel (2-layer GCN + image-query matmul).

Math (reference):
    norm = dinv[src] * w * dinv[dst],  dinv = rsqrt(segment_sum(w, dst))
    x1 = leaky_relu(segsum(norm * (NF @ W1)[src], dst) + b1, 0.2)
    x2 = segsum(norm * (x1 @ W2)[src], dst) + b2
    out = img @ x2.T                                  # [64, 20000]

Algebraic restructure (exact up to fp reassociation), with D = diag(dinv)
and A_w the weighted adjacency:
    aggF = D A_w (D NF)        -- src-side D pre-folded into the NF table,
                                  dst-side D applied per dst-node column
    x1T  = lrelu(W1.T @ aggF.T + b1)                  # [HID, N]
    PT   = W2 @ imgT                                  # [HID, B]
    Qs   = D (x1T.T @ PT)                             # [N, B] pre-scaled
    out  = (A_w Qs scaled by D on dst).T + img @ b2

Sharding: nodes (and their incoming edges) range-sharded across 8 cores;
segment sums are fully core-local (PE matmuls against one-hot selection
matrices S1 = (iota == dst_local) * w, built once in L1 and stored).
Three SPMD launches:
  L1: S1 matrices + deg -> dinv (vectorized across blocks); the bf16
      dinv-scaled NF gather table; per-core partial PT over a 208-col
      slice of W2; partial img@b2.
  L2: gather NF-table rows by edge src, aggregate via S1, dst-scale,
      x1T = lrelu(...), Qs = dinv * (x1 @ PT); phases interleaved per
      500-node chunk so gathers overlap the x1/Q matmuls.
  L3: gather Qs rows by src, aggregate via S1, dst-scale, + img@b2
Host work between launches is pure layout (concat / transpose of shards).
"""

from contextlib import nullcontext

import numpy as np

from concourse import bacc, bass, mybir
from concourse.bass_utils import run_bass_kernel_spmd
from concourse.masks import make_identity
from concourse.tile import TileContext


def _maybe_reps(tc, reps):
    """Hardware repeat loop for timing (reps>1); no-op for production."""
    return tc.For_i(0, reps) if reps > 1 else nullcontext()

P = 128
NB = 125            # nodes per block (psum free dim)
F_TEXT = 300
FPAD = 320          # W1 rows padded (300 -> 320 = 128+128+64 chunks)
NFS_W = 384         # bf16 NF-table row: 384*2B = 768B (256B multiple)
QS_W = 128          # bf16 Qs-table row: 128*2B = 256B
NEG = 0.2
FCH = ((0, P), (P, P), (2 * P, 64))   # feature chunks of FPAD

# full-size problem config
CFG_FULL = dict(B=64, N=20000, E=160000, HID=1024, OUT=1664, CORES=8)

TRACE = False                  # set by test.py for profiling
LAST_EXEC_NS = {}              # launch name -> exec ns (when TRACE)
LAST_BUILD = None              # (nc1, nc2, nc3) from the last kernel() call
LAST_MAPS = None               # {"l1": maps1, "l2": maps2, "l3": maps3}
LAST_REP_BUILDERS = None       # launch name -> (lambda reps: nc), for timing
LAUNCH_ORDER = ["l1", "l2", "l3"]

_BUILD_CACHE = {}


# ----------------------------------------------------------------- host prep

def _prep_edges(edge_src, edge_dst, edge_weight, cfg):
    """Group edges by (core, block) of their dst; pad each block's edge list
    to T_b*128 where T_b is the max tile count for block index b across
    cores (SPMD needs identical program structure on every core)."""
    ncores = cfg["CORES"]
    npc = cfg["N"] // ncores               # nodes per core
    nblk = npc // NB                       # blocks per core
    assert npc % NB == 0

    core = edge_dst // npc
    blk = (edge_dst - core * npc) // NB
    dstl = (edge_dst - core * npc) - blk * NB

    buckets = [[None] * nblk for _ in range(ncores)]
    order = np.lexsort((blk, core))
    core_s, blk_s = core[order], blk[order]
    bounds = np.searchsorted(core_s * nblk + blk_s, np.arange(ncores * nblk + 1))
    for k in range(ncores):
        for b in range(nblk):
            i0, i1 = bounds[k * nblk + b], bounds[k * nblk + b + 1]
            buckets[k][b] = order[i0:i1]

    TBs = []
    for b in range(nblk):
        mx = max(len(buckets[k][b]) for k in range(ncores))
        TBs.append(max(1, -(-mx // P)))

    per_core = []
    for k in range(ncores):
        srcs, dls, ws = [], [], []
        for b in range(nblk):
            ids = buckets[k][b]
            pad = TBs[b] * P - len(ids)
            srcs.append(np.pad(edge_src[ids], (0, pad)))
            dls.append(np.pad(dstl[ids], (0, pad)))
            ws.append(np.pad(edge_weight[ids], (0, pad)))
        src = np.concatenate(srcs).astype(np.int64)
        dl = np.concatenate(dls).astype(np.float32)
        w = np.concatenate(ws).astype(np.float32)

        def idx16(a):
            # dma_gather layout: idx j at [j%16, j//16], replicated on 8
            # 16-partition groups -> [128, n/16]
            a16 = a.astype(np.int16).reshape(-1, 16).T
            return np.tile(a16, (8, 1)).copy()

        per_core.append(dict(
            src16=idx16(src),
            dstl=dl.reshape(-1, P).T.copy(),     # [128, TT]
            wts=w.reshape(-1, P).T.copy(),       # [128, TT]
        ))
    return TBs, per_core


# ------------------------------------------------------------------ builders

def _new_nc():
    return bacc.Bacc(None, target_bir_lowering=False)


def _iota_row(nc, pool):
    """[128, NB] f32 tile with value j in column j (every partition)."""
    ji = pool.tile([P, NB], mybir.dt.int32)
    nc.gpsimd.iota(ji[:], pattern=[[1, NB]], base=0, channel_multiplier=0)
    j = pool.tile([P, NB], mybir.dt.float32)
    nc.vector.tensor_copy(j[:], ji[:])
    return j


def _build_l1(TBs, cfg, reps=1):
    """S1 matrices; deg -> dinv; bf16 dinv*NF table; partial PT."""
    nc = _new_nc()
    nblk = len(TBs)
    TT = sum(TBs)
    N, B, HID, OUT = cfg["N"], cfg["B"], cfg["HID"], cfg["OUT"]
    npc = N // cfg["CORES"]
    OSL = OUT // cfg["CORES"]              # 208 = 128 + 80
    HCH = HID // P

    dstl_in = nc.dram_tensor("dstl", [P, TT], mybir.dt.float32, kind="ExternalInput")
    wts_in = nc.dram_tensor("wts", [P, TT], mybir.dt.float32, kind="ExternalInput")
    nfsl_in = nc.dram_tensor("nfslab", [npc, F_TEXT], mybir.dt.float32,
                             kind="ExternalInput")
    w2ts_in = nc.dram_tensor("w2ts", [OSL, HID], mybir.dt.float32,
                             kind="ExternalInput")
    imgts_in = nc.dram_tensor("imgts", [OSL, B], mybir.dt.float32,
                              kind="ExternalInput")
    b2s_in = nc.dram_tensor("b2s", [OSL, 1], mybir.dt.float32,
                            kind="ExternalInput")

    nfs_out = nc.dram_tensor("nfs", [npc, NFS_W], mybir.dt.bfloat16,
                             kind="ExternalOutput")
    s1_out = nc.dram_tensor("s1", [P, TT * NB], mybir.dt.bfloat16,
                            kind="ExternalOutput")
    dv2d_out = nc.dram_tensor("dv2d", [NB, nblk], mybir.dt.float32,
                              kind="ExternalOutput")
    dvr_out = nc.dram_tensor("dvrow", [1, npc], mybir.dt.float32,
                             kind="ExternalOutput")
    ptp_out = nc.dram_tensor("ptp", [HID, B], mybir.dt.float32,
                             kind="ExternalOutput")
    cvp_out = nc.dram_tensor("cvp", [B, 1], mybir.dt.float32,
                             kind="ExternalOutput")

    with TileContext(nc) as tc:
        with (
            tc.tile_pool(name="sbA", bufs=1) as sbA,
            tc.tile_pool(name="sbS", bufs=4) as sbS,
            tc.tile_pool(name="ps", bufs=2, space="PSUM") as ps,
        ):
            with _maybe_reps(tc, reps):
                J = _iota_row(nc, sbA)
                identNB = sbA.tile([NB, NB], mybir.dt.float32)
                make_identity(nc, identNB[:])
                ones_bf = sbA.tile([P, 1], mybir.dt.bfloat16)
                nc.vector.memset(ones_bf[:], 1.0)

                dstl = sbA.tile([P, TT], mybir.dt.float32)
                wts = sbA.tile([P, TT], mybir.dt.float32)
                nc.scalar.dma_start(out=dstl[:], in_=dstl_in[:])
                nc.scalar.dma_start(out=wts[:], in_=wts_in[:])

                # ---- partial PT = W2[:, oslice] @ imgT[oslice], img@b2 -----
                w2a = sbA.tile([P, HID], mybir.dt.float32)
                w2b = sbA.tile([OSL - P, HID], mybir.dt.float32)
                nc.scalar.dma_start(out=w2a[:], in_=w2ts_in[0:P, :])
                nc.scalar.dma_start(out=w2b[:], in_=w2ts_in[P:OSL, :])
                imga = sbA.tile([P, B], mybir.dt.float32)
                imgb = sbA.tile([OSL - P, B], mybir.dt.float32)
                nc.scalar.dma_start(out=imga[:], in_=imgts_in[0:P, :])
                nc.scalar.dma_start(out=imgb[:], in_=imgts_in[P:OSL, :])
                b2a = sbA.tile([P, 1], mybir.dt.float32)
                b2b = sbA.tile([OSL - P, 1], mybir.dt.float32)
                nc.scalar.dma_start(out=b2a[:], in_=b2s_in[0:P, :])
                nc.scalar.dma_start(out=b2b[:], in_=b2s_in[P:OSL, :])

                ptall = sbA.tile([P, HCH, B], mybir.dt.float32)
                for h in range(HCH):
                    ptps = ps.tile([P, B], mybir.dt.float32, space="PSUM",
                                   tag="pt")
                    nc.tensor.matmul(out=ptps[:], lhsT=w2a[:, h * P:(h + 1) * P],
                                     rhs=imga[:], start=True, stop=False)
                    nc.tensor.matmul(out=ptps[:], lhsT=w2b[:, h * P:(h + 1) * P],
                                     rhs=imgb[:], start=False, stop=True)
                    nc.vector.tensor_copy(ptall[:, h, :], ptps[:])
                nc.sync.dma_start(
                    out=bass.AP(ptp_out, 0, [[B, P], [P * B, HCH], [1, B]]),
                    in_=ptall[:])
                cvps = ps.tile([B, 1], mybir.dt.float32, space="PSUM", tag="cv")
                nc.tensor.matmul(out=cvps[:], lhsT=imga[:], rhs=b2a[:],
                                 start=True, stop=False)
                nc.tensor.matmul(out=cvps[:], lhsT=imgb[:], rhs=b2b[:],
                                 start=False, stop=True)
                cvsb = sbS.tile([B, 1], mybir.dt.float32, tag="cvsb")
                nc.vector.tensor_copy(cvsb[:], cvps[:])
                nc.sync.dma_start(out=cvp_out[:], in_=cvsb[:])

                # ---- S1 + deg for all blocks ------------------------------
                s1sb = sbA.tile([P, TT * NB], mybir.dt.bfloat16)
                degall = ps.tile([NB, nblk], mybir.dt.float32, space="PSUM",
                                 tag="deg", bufs=1)
                half = nblk // 2
                toff = 0
                for b in range(nblk):
                    Tb = TBs[b]
                    for t in range(Tb):
                        col = toff + t
                        s1t = s1sb[:, col * NB:(col + 1) * NB]
                        nc.vector.tensor_scalar(
                            out=s1t, in0=J[:],
                            scalar1=dstl[:, col:col + 1],
                            scalar2=wts[:, col:col + 1],
                            op0=mybir.AluOpType.is_equal,
                            op1=mybir.AluOpType.mult)
                        nc.tensor.matmul(out=degall[:, b:b + 1], lhsT=s1t,
                                         rhs=ones_bf[:],
                                         start=(t == 0), stop=(t == Tb - 1))
                    toff += Tb
                    if b == half - 1:
                        mid = toff
                        nc.sync.dma_start(out=s1_out[:, 0:mid * NB],
                                          in_=s1sb[:, 0:mid * NB])
                nc.sync.dma_start(out=s1_out[:, mid * NB:TT * NB],
                                  in_=s1sb[:, mid * NB:TT * NB])

                # ---- dinv (vectorized over all blocks) --------------------
                m = sbS.tile([NB, nblk], mybir.dt.float32, tag="m")
                nc.vector.tensor_scalar(out=m[:], in0=degall[:], scalar1=0.0,
                                        scalar2=None, op0=mybir.AluOpType.is_gt)
                safe = sbS.tile([NB, nblk], mybir.dt.float32, tag="safe")
                nc.vector.tensor_scalar(out=safe[:], in0=degall[:],
                                        scalar1=1e-30, scalar2=None,
                                        op0=mybir.AluOpType.max)
                rec = sbS.tile([NB, nblk], mybir.dt.float32, tag="rec")
                nc.vector.reciprocal(rec[:], safe[:])
                sq = sbS.tile([NB, nblk], mybir.dt.float32, tag="sq")
                nc.scalar.sqrt(sq[:], rec[:])
                dvsb = sbA.tile([NB, nblk], mybir.dt.float32)
                nc.vector.tensor_tensor(out=dvsb[:], in0=sq[:], in1=m[:],
                                        op=mybir.AluOpType.mult)
                nc.sync.dma_start(out=dv2d_out[:], in_=dvsb[:])
                # dvrow[0, b*NB:(b+1)*NB] = dinv of block b (row layout on
                # partition 0, so matmul rhs base-partition stays 0)
                dvrsb = sbA.tile([1, npc], mybir.dt.float32)
                for b in range(nblk):
                    rps = ps.tile([1, NB], mybir.dt.float32, space="PSUM",
                                  tag="dvr")
                    nc.tensor.matmul(out=rps[:], lhsT=dvsb[:, b:b + 1],
                                     rhs=identNB[:], start=True, stop=True)
                    nc.vector.tensor_copy(dvrsb[:, b * NB:(b + 1) * NB],
                                          rps[:])
                nc.sync.dma_start(out=dvr_out[:], in_=dvrsb[:])

                # ---- bf16 dinv-scaled NF table ----------------------------
                nfbsb = sbA.tile([NB, nblk, F_TEXT], mybir.dt.float32)
                nc.scalar.dma_start(
                    out=nfbsb[:],
                    in_=bass.AP(nfsl_in, 0,
                                [[F_TEXT, NB], [NB * F_TEXT, nblk],
                                 [1, F_TEXT]]))
                nfssb = sbA.tile([NB, nblk, NFS_W], mybir.dt.bfloat16)
                nc.vector.memset(nfssb[:, :, F_TEXT:NFS_W], 0.0)
                for b in range(nblk):
                    nc.vector.tensor_scalar(
                        out=nfssb[:, b, 0:F_TEXT], in0=nfbsb[:, b, :],
                        scalar1=dvsb[:, b:b + 1], scalar2=None,
                        op0=mybir.AluOpType.mult)
                nc.sync.dma_start(
                    out=bass.AP(nfs_out, 0,
                                [[NFS_W, NB], [NB * NFS_W, nblk], [1, NFS_W]]),
                    in_=nfssb[:])
    nc.finalize()
    return nc


def _build_l2(TBs, cfg, reps=1):
    """Layer-1 aggregation + x1T + Qs table, interleaved per 500-node chunk."""
    nc = _new_nc()
    nblk = len(TBs)
    TT = sum(TBs)
    N, B, HID, OUT = cfg["N"], cfg["B"], cfg["HID"], cfg["OUT"]
    ncores = cfg["CORES"]
    npc = N // ncores
    HCH = HID // P          # 8
    NCW = 500               # x1T n-chunk width (f32r full-rate needs >=256)
    NCH = npc // NCW        # 5
    NCHB = NCW // NB        # 4 blocks per chunk

    nfs_in = nc.dram_tensor("nfs", [N, NFS_W], mybir.dt.bfloat16,
                            kind="ExternalInput")
    s1_in = nc.dram_tensor("s1", [P, TT * NB], mybir.dt.bfloat16,
                           kind="ExternalInput")
    src_in = nc.dram_tensor("src16", [P, TT * 8], mybir.dt.int16,
                            kind="ExternalInput")
    dv2d_in = nc.dram_tensor("dv2d", [NB, nblk], mybir.dt.float32,
                             kind="ExternalInput")
    dvr_in = nc.dram_tensor("dvrow", [1, npc], mybir.dt.float32,
                            kind="ExternalInput")
    ptp_in = nc.dram_tensor("ptp", [HID, ncores * B], mybir.dt.float32,
                            kind="ExternalInput")
    w1_in = nc.dram_tensor("w1", [FPAD, HID], mybir.dt.float32r,
                           kind="ExternalInput")
    b1c_in = nc.dram_tensor("b1c", [P, HCH], mybir.dt.float32,
                            kind="ExternalInput")

    qs_out = nc.dram_tensor("qs", [npc, QS_W], mybir.dt.bfloat16,
                            kind="ExternalOutput")

    with TileContext(nc) as tc:
        with (
            tc.tile_pool(name="sbA", bufs=1) as sbA,
            tc.tile_pool(name="sbC", bufs=2) as sbC,
            tc.tile_pool(name="sbG", bufs=3) as sbG,
            tc.tile_pool(name="sbS", bufs=4) as sbS,
            tc.tile_pool(name="ps1", bufs=1, space="PSUM") as ps1,
            tc.tile_pool(name="ps2", bufs=2, space="PSUM") as ps2,
        ):
            with _maybe_reps(tc, reps):
                s1_all = sbA.tile([P, TT * NB], mybir.dt.bfloat16)
                src16 = sbA.tile([P, TT * 8], mybir.dt.int16)
                dv2d = sbA.tile([NB, nblk], mybir.dt.float32)
                dvrow = sbA.tile([1, npc], mybir.dt.float32)
                b1c = sbA.tile([P, HCH], mybir.dt.float32)
                nc.scalar.dma_start(out=s1_all[:], in_=s1_in[:])
                nc.sync.dma_start(out=src16[:], in_=src_in[:])
                nc.sync.dma_start(out=dv2d[:], in_=dv2d_in[:])
                nc.sync.dma_start(out=dvrow[:], in_=dvr_in[:])
                nc.sync.dma_start(out=b1c[:], in_=b1c_in[:])
                ones_row = sbA.tile([1, P], mybir.dt.float32)
                nc.vector.memset(ones_row[:], 1.0)
                alph = sbA.tile([P, 1], mybir.dt.float32)
                nc.vector.memset(alph[:], NEG)

                # ---- PT = sum of the 8 per-core partials; bf16 ------------
                PTf = sbA.tile([P, HCH * B], mybir.dt.float32)
                with tc.tile_pool(name="sbE", bufs=1) as sbE:
                    ptsb = sbE.tile([P, HCH, ncores * B], mybir.dt.float32)
                    nc.scalar.dma_start(
                        out=ptsb[:],
                        in_=bass.AP(ptp_in, 0,
                                    [[ncores * B, P], [P * ncores * B, HCH],
                                     [1, ncores * B]]))
                    for h in range(HCH):
                        acc = PTf[:, h * B:(h + 1) * B]
                        nc.vector.tensor_copy(acc, ptsb[:, h, 0:B])
                        for j in range(1, ncores):
                            nc.vector.tensor_tensor(
                                out=acc, in0=acc,
                                in1=ptsb[:, h, j * B:(j + 1) * B],
                                op=mybir.AluOpType.add)
                PT = sbA.tile([P, HCH * B], mybir.dt.bfloat16)
                nc.vector.tensor_copy(PT[:], PTf[:])

                w1t = [sbA.tile([P, HID], mybir.dt.float32r, name="w1k0"),
                       sbA.tile([P, HID], mybir.dt.float32r, name="w1k1"),
                       sbA.tile([64, HID], mybir.dt.float32r, name="w1k2")]
                nc.scalar.dma_start(out=w1t[0][:], in_=w1_in[0:P, :])
                nc.scalar.dma_start(out=w1t[1][:], in_=w1_in[P:2 * P, :])
                nc.scalar.dma_start(out=w1t[2][:], in_=w1_in[2 * P:2 * P + 64, :])

                # ---- interleaved chunks: gather/agg -> x1 -> Qs -----------
                btoffs = np.concatenate([[0], np.cumsum(TBs)]).tolist()
                for nchi in range(NCH):
                    aggc = [sbC.tile([P, NCW], mybir.dt.float32r,
                                     tag="aggc0", bufs=2, name="aggc0"),
                            sbC.tile([P, NCW], mybir.dt.float32r,
                                     tag="aggc1", bufs=2, name="aggc1"),
                            sbC.tile([64, NCW], mybir.dt.float32r,
                                     tag="aggc2", bufs=2, name="aggc2")]
                    for bb in range(NCHB):
                        blk = nchi * NCHB + bb
                        Tb = TBs[blk]
                        toff = btoffs[blk]
                        ni = Tb * P
                        nfg = sbG.tile([P, Tb, NFS_W], mybir.dt.bfloat16,
                                       tag="nfg")
                        nc.gpsimd.dma_gather(
                            out_ap=nfg[:], in_ap=nfs_in[:],
                            idxs_ap=src16[:, toff * 8:(toff + Tb) * 8],
                            num_idxs=ni, num_idxs_reg=ni, elem_size=NFS_W,
                            single_packet=False)
                        # pa0 | pa1 | pa2 | dvb packed in one psum bank
                        pall = ps1.tile([P, 4 * NB], mybir.dt.float32,
                                        space="PSUM", tag="pa", bufs=2)
                        pa = [pall[:, 0:NB], pall[:, NB:2 * NB],
                              pall[0:64, 2 * NB:3 * NB]]
                        dvb = pall[:, 3 * NB:4 * NB]
                        # One psum bank holds pa0|pa1|pa2|dvb.  start=True
                        # clears has_written for the WHOLE bank, so only the
                        # very first matmul into the bank may set it; later
                        # regions first-write with start=False (overwrite on
                        # clear bit) and then accumulate.
                        for fc, (f0, fw) in enumerate(FCH):
                            for t in range(Tb):
                                s1t = s1_all[:,
                                             (toff + t) * NB:(toff + t + 1) * NB]
                                nc.tensor.matmul(
                                    out=pa[fc],
                                    lhsT=nfg[:, t, f0:f0 + fw], rhs=s1t,
                                    start=(t == 0 and fc == 0),
                                    stop=(t == Tb - 1))
                        nc.tensor.matmul(
                            out=dvb, lhsT=ones_row[:],
                            rhs=dvrow[:, blk * NB:(blk + 1) * NB],
                            start=False, stop=True)
                        dvbs = sbS.tile([P, NB], mybir.dt.float32,
                                        tag="dvbs")
                        nc.vector.tensor_copy(dvbs[:], dvb)
                        for fc, (f0, fw) in enumerate(FCH):
                            nc.vector.tensor_tensor(
                                out=aggc[fc][:, bb * NB:(bb + 1) * NB],
                                in0=pa[fc], in1=dvbs[0:fw, :],
                                op=mybir.AluOpType.mult)
                    x1c = [sbC.tile([P, NCW], mybir.dt.bfloat16,
                                    tag=f"x1c{h}", bufs=2, name=f"x1c{h}")
                           for h in range(HCH)]
                    for h in range(HCH):
                        px = ps2.tile([P, NCW], mybir.dt.float32, space="PSUM",
                                      tag="px", bufs=2)
                        for kc in range(3):
                            nc.tensor.matmul(
                                out=px[:],
                                lhsT=w1t[kc][:, h * P:(h + 1) * P],
                                rhs=aggc[kc][:, :],
                                start=(kc == 0), stop=(kc == 2))
                        nc.scalar.activation(
                            out=x1c[h][:], in_=px[:],
                            func=mybir.ActivationFunctionType.Prelu,
                            bias=b1c[:, h:h + 1], alpha=alph[:])
                    for bb in range(NCHB):
                        blk = nchi * NCHB + bb
                        n0 = blk * NB
                        pq = ps2.tile([NB, B], mybir.dt.float32, space="PSUM",
                                      tag="pq", bufs=2)
                        for h in range(HCH):
                            nc.tensor.matmul(
                                out=pq[:], lhsT=x1c[h][:, bb * NB:(bb + 1) * NB],
                                rhs=PT[:, h * B:(h + 1) * B],
                                start=(h == 0), stop=(h == HCH - 1))
                        qsb = sbS.tile([NB, QS_W], mybir.dt.bfloat16, tag="qsb")
                        nc.vector.memset(qsb[:, B:QS_W], 0.0)
                        nc.vector.tensor_scalar(
                            out=qsb[:, 0:B], in0=pq[:],
                            scalar1=dv2d[:, blk:blk + 1], scalar2=None,
                            op0=mybir.AluOpType.mult)
                        eng = nc.sync if blk % 2 == 0 else nc.scalar
                        eng.dma_start(out=qs_out[n0:n0 + NB, :], in_=qsb[:])
    nc.finalize()
    return nc


def _build_l3(TBs, cfg, reps=1):
    """Layer-2 aggregation over Qs -> output shard [B, npc]."""
    nc = _new_nc()
    nblk = len(TBs)
    TT = sum(TBs)
    N, B = cfg["N"], cfg["B"]
    ncores = cfg["CORES"]
    npc = N // ncores

    qs_in = nc.dram_tensor("qs", [N, QS_W], mybir.dt.bfloat16,
                           kind="ExternalInput")
    s1_in = nc.dram_tensor("s1", [P, TT * NB], mybir.dt.bfloat16,
                           kind="ExternalInput")
    src_in = nc.dram_tensor("src16", [P, TT * 8], mybir.dt.int16,
                            kind="ExternalInput")
    dvr_in = nc.dram_tensor("dvrow", [1, npc], mybir.dt.float32,
                            kind="ExternalInput")
    cvp_in = nc.dram_tensor("cvp", [B, ncores], mybir.dt.float32,
                            kind="ExternalInput")
    out_own = nc.dram_tensor("outp", [B, npc], mybir.dt.float32,
                             kind="ExternalOutput")

    with TileContext(nc) as tc:
        with (
            tc.tile_pool(name="sbA", bufs=1) as sbA,
            tc.tile_pool(name="sbG", bufs=3) as sbG,
            tc.tile_pool(name="sbS", bufs=4) as sbS,
            tc.tile_pool(name="ps", bufs=2, space="PSUM") as ps,
        ):
            with _maybe_reps(tc, reps):
                s1_all = sbA.tile([P, TT * NB], mybir.dt.bfloat16)
                src16 = sbA.tile([P, TT * 8], mybir.dt.int16)
                dvrow = sbA.tile([1, npc], mybir.dt.float32)
                cvp = sbA.tile([B, ncores], mybir.dt.float32)
                nc.scalar.dma_start(out=s1_all[:], in_=s1_in[:])
                nc.sync.dma_start(out=src16[:], in_=src_in[:])
                nc.sync.dma_start(out=dvrow[:], in_=dvr_in[:])
                nc.sync.dma_start(out=cvp[:], in_=cvp_in[:])
                c_sb = sbA.tile([B, 1], mybir.dt.float32)
                nc.vector.tensor_reduce(out=c_sb[:], in_=cvp[:],
                                        axis=mybir.AxisListType.X,
                                        op=mybir.AluOpType.add)
                ones_row = sbA.tile([1, B], mybir.dt.float32)
                nc.vector.memset(ones_row[:], 1.0)
                outsb = sbA.tile([B, npc], mybir.dt.float32)

                toff = 0
                for b in range(nblk):
                    Tb = TBs[b]
                    ni = Tb * P
                    qg = sbG.tile([P, Tb, QS_W], mybir.dt.bfloat16, tag="qg")
                    nc.gpsimd.dma_gather(
                        out_ap=qg[:], in_ap=qs_in[:],
                        idxs_ap=src16[:, toff * 8:(toff + Tb) * 8],
                        num_idxs=ni, num_idxs_reg=ni, elem_size=QS_W,
                        single_packet=False)
                    # po | dvb packed in one psum bank
                    pall = ps.tile([B, 2 * NB], mybir.dt.float32, space="PSUM",
                                   tag="po", bufs=2)
                    po, dvb = pall[:, 0:NB], pall[:, NB:2 * NB]
                    for t in range(Tb):
                        s1t = s1_all[:, (toff + t) * NB:(toff + t + 1) * NB]
                        nc.tensor.matmul(out=po, lhsT=qg[:, t, 0:B], rhs=s1t,
                                         start=(t == 0), stop=(t == Tb - 1))
                    nc.tensor.matmul(out=dvb, lhsT=ones_row[:],
                                     rhs=dvrow[:, b * NB:(b + 1) * NB],
                                     start=True, stop=True)
                    dvbs = sbS.tile([B, NB], mybir.dt.float32,
                                    tag="dvbs")
                    nc.vector.tensor_copy(dvbs[:], dvb)
                    osl = outsb[:, b * NB:(b + 1) * NB]
                    nc.vector.tensor_tensor(out=osl, in0=po, in1=dvbs[:],
                                            op=mybir.AluOpType.mult)
                    nc.vector.tensor_scalar(out=osl, in0=osl,
                                            scalar1=c_sb[:, 0:1], scalar2=None,
                                            op0=mybir.AluOpType.add)
                    toff += Tb
                nc.sync.dma_start(out=out_own[:], in_=outsb[:])
    nc.finalize()
    return nc


# ------------------------------------------------------------------- runner

def _run(name, nc, in_maps, cores):
    kw = {}
    if TRACE:
        kw = dict(trace=True)
    res = run_bass_kernel_spmd(nc, in_maps, core_ids=list(range(cores)), **kw)
    if res.exec_time_ns is not None:
        LAST_EXEC_NS[name] = res.exec_time_ns
    return res.results


def _kernel_impl(img_feat, node_features, edge_src, edge_dst, edge_weight,
                 W1, b1, W2, b2, cfg):
    ncores = cfg["CORES"]
    N, B, HID, OUT = cfg["N"], cfg["B"], cfg["HID"], cfg["OUT"]
    npc = N // ncores
    OSL = OUT // ncores
    HCH = HID // P

    TBs, per_core = _prep_edges(edge_src, edge_dst, edge_weight, cfg)
    key = (tuple(TBs), tuple(sorted(cfg.items())))
    if key not in _BUILD_CACHE:
        _BUILD_CACHE[key] = (_build_l1(TBs, cfg), _build_l2(TBs, cfg),
                             _build_l3(TBs, cfg))
    nc1, nc2, nc3 = _BUILD_CACHE[key]

    # ---- L1
    W2T = np.ascontiguousarray(W2.T).astype(np.float32)        # [OUT, HID]
    imgT = np.ascontiguousarray(img_feat.T).astype(np.float32)  # [OUT, B]
    maps1 = [dict(dstl=pc["dstl"], wts=pc["wts"],
                  nfslab=np.ascontiguousarray(
                      node_features[k * npc:(k + 1) * npc]).astype(np.float32),
                  w2ts=np.ascontiguousarray(W2T[k * OSL:(k + 1) * OSL]),
                  imgts=np.ascontiguousarray(imgT[k * OSL:(k + 1) * OSL]),
                  b2s=np.ascontiguousarray(
                      b2[k * OSL:(k + 1) * OSL].reshape(-1, 1)).astype(
                          np.float32))
             for k, pc in enumerate(per_core)]
    r1 = _run("l1", nc1, maps1, ncores)
    nfs = np.concatenate([r1[k]["nfs"] for k in range(ncores)], axis=0)
    ptp = np.concatenate([r1[k]["ptp"] for k in range(ncores)], axis=1)
    cvp = np.concatenate([r1[k]["cvp"] for k in range(ncores)], axis=1)

    # ---- L2
    w1_pad = np.zeros((FPAD, HID), np.float32)
    w1_pad[:F_TEXT, :] = W1
    b1c = np.ascontiguousarray(
        np.asarray(b1, np.float32).reshape(HCH, P).T)           # [128, 8]
    maps2 = [dict(nfs=nfs, s1=r1[k]["s1"], src16=pc["src16"],
                  dv2d=r1[k]["dv2d"], dvrow=r1[k]["dvrow"],
                  ptp=ptp, w1=w1_pad, b1c=b1c)
             for k, pc in enumerate(per_core)]
    r2 = _run("l2", nc2, maps2, ncores)
    qs = np.concatenate([r2[k]["qs"] for k in range(ncores)], axis=0)

    # ---- L3
    maps3 = [dict(qs=qs, s1=r1[k]["s1"], src16=pc["src16"],
                  dvrow=r1[k]["dvrow"], cvp=cvp)
             for k, pc in enumerate(per_core)]
    r3 = _run("l3", nc3, maps3, ncores)
    out = np.concatenate([r3[k]["outp"] for k in range(ncores)], axis=1)

    global LAST_BUILD, LAST_MAPS, LAST_REP_BUILDERS
    LAST_BUILD = (nc1, nc2, nc3)
    LAST_MAPS = {"l1": maps1, "l2": maps2, "l3": maps3}
    LAST_REP_BUILDERS = {
        "l1": lambda reps: _build_l1(TBs, cfg, reps=reps),
        "l2": lambda reps: _build_l2(TBs, cfg, reps=reps),
        "l3": lambda reps: _build_l3(TBs, cfg, reps=reps),
    }
    return out.astype(np.float32)


def kernel(img_feat, node_features, edge_src, edge_dst, edge_weight,
           W1, b1, W2, b2):
    return _kernel_impl(np.asarray(img_feat), np.asarray(node_features),
                        np.asarray(edge_src), np.asarray(edge_dst),
                        np.asarray(edge_weight), np.asarray(W1),
                        np.asarray(b1), np.asarray(W2), np.asarray(b2),
                        CFG_FULL)
